# revision 15
# baseline (speedup 1.0000x reference)
"""Trainium2 Bass kernel for the sparse_attention nn.Module problem.

Strategy: data-parallel over the MSA-row dim S (S=128 -> 16 rows per core,
8 cores). All projection weights + pair bias replicated; mask bias and
activations sharded with S. No collectives.

Design (vs the 298us v1 baseline):
  - Scores matmuls (K=DH=32) use 4x PE row-tiling (tile_position=(32*hh,0)):
    the 4 heads of a head-group run concurrently in 32-row bands of the PE
    array, consuming qT/kT in their natural [(hh,d), ...] projection layout
    (no DMA remap). The 4 concurrent tiles land in 4 distinct PSUM banks,
    split as two 2-bank tiles (hh01/hh23, pool bufs=2) so exp() releases
    banks in halves and the next group's matmuls never stall on a
    whole-tile WAR.
  - AV and softmax-denominator (Z) matmuls use 4x PE column-tiling
    (tile_position=(0,32*hh)), producing o and Z TRANSPOSED: [(hh,d),(tc,q)].
    This kills the PE transpose of the gated output: og in [t,q] layout is
    directly the lhsT of the final projection. kc is the inner loop of each
    accumulation (a PSUM bank tolerates one pending accumulation group).
  - q/k projections batch TWO rows per matmul (N=512, shared weights),
    halving their LDWEIGHTS+MATMUL count (the PE is LDW-count-bound).
  - The gate is computed transposed (gT = Wg @ x^T) so bg rides as the ACT
    per-partition bias; sigmoid(x)=0.5*(tanh(x/2)+1) with 0.5 folded into Wo.
  - exp(s+mask) as FD=1024 ACTIVATEs (mask per-partition, partitions=keys);
    exp(pair) is a host-precomputed resident tile applied with bf16 2x DVE
    multiplies; 1/Z via reciprocal_approx_fast; bo via a K=1 rank-1 matmul;
    final out evicted fp16 by ScalarE; GPSIMD off the critical path.
"""

import os
import numpy as np
import ml_dtypes

def _mmdt():
    return (ml_dtypes.bfloat16 if os.environ.get('KDTYPE', 'fp16') == 'bf16'
            else np.float16)

B, S, Q, C = 1, 128, 256, 256
H, DH = 8, 32
TOT = H * DH
N_CORES = 8
S_LOC = S // N_CORES  # 16

_CACHE = {}


def _build_program(s_loc, bg_const=None):
    import concourse.bacc as bacc
    import concourse.mybir as mybir
    from concourse import tile
    from concourse.alu_op_type import AluOpType as ALU

    assert s_loc % 2 == 0
    dt = mybir.dt
    f32, bf16 = dt.float32, dt.bfloat16
    f16 = bf16 if os.environ.get('KDTYPE', 'fp16') == 'bf16' else dt.float16
    AF = mybir.ActivationFunctionType

    nc = bacc.Bacc("TRN2", target_bir_lowering=False, debug=False,
                   num_devices=N_CORES)

    x_d = nc.dram_tensor("x", [s_loc, 2 * C, Q], f16, kind="ExternalInput").ap()
    mask_d = nc.dram_tensor("maskt", [128, 2 * s_loc], f32, kind="ExternalInput").ap()
    expb_d = nc.dram_tensor("expb", [128, 2 * H * Q], bf16, kind="ExternalInput").ap()
    wq_d = nc.dram_tensor("wq", [128, 512], f16, kind="ExternalInput").ap()
    wk_d = nc.dram_tensor("wk", [128, 512], f16, kind="ExternalInput").ap()
    wv_d = nc.dram_tensor("wv", [128, 512], f16, kind="ExternalInput").ap()
    wg_d = nc.dram_tensor("wg", [128, 512], f16, kind="ExternalInput").ap()
    wo_d = nc.dram_tensor("wo", [128, 512], f16, kind="ExternalInput").ap()
    bgt_d = nc.dram_tensor("bgt", [128, 2], f32, kind="ExternalInput").ap()
    bo_d = nc.dram_tensor("bo", [1, 256], f16, kind="ExternalInput").ap()
    ones1_d = nc.dram_tensor("ones1", [1, 128], f16, kind="ExternalInput").ap()
    ones32_d = nc.dram_tensor("ones32", [128, 32], bf16, kind="ExternalInput").ap()
    out_d = nc.dram_tensor("out", [s_loc, Q, C], f16, kind="ExternalOutput").ap()

    with tile.TileContext(nc) as tc:
        with (
            tc.tile_pool(name="const", bufs=1) as cp,
            tc.tile_pool(name="work", bufs=3) as wp,
            tc.tile_pool(name="ps_small", bufs=2, space="PSUM") as pps,
            tc.tile_pool(name="ps_sc", bufs=2, space="PSUM") as psc,
            tc.tile_pool(name="ps_o", bufs=1, space="PSUM") as pso,
            tc.tile_pool(name="ps_z", bufs=1, space="PSUM") as psz,
        ):
            # ---- resident constants ----
            wq_t = cp.tile([128, 512], f16, tag="wq")
            wk_t = cp.tile([128, 512], f16, tag="wk")
            wv_t = cp.tile([128, 512], f16, tag="wv")
            wg_t = cp.tile([128, 512], f16, tag="wg")
            wo_t = cp.tile([128, 512], f16, tag="wo")
            expb_t = cp.tile([128, 2 * H * Q], bf16, tag="expb")
            mask_t = cp.tile([128, 2 * s_loc], f32, tag="mask")
            bgt_t = cp.tile([128, 2], f32, tag="bgt")
            bo_t = cp.tile([1, 256], f16, tag="bo")
            ones1_t = cp.tile([1, 128], f16, tag="ones1")
            ones32_t = cp.tile([128, 32], bf16, tag="ones32")


            for sp in range(s_loc // 2):
                s0 = 2 * sp
                # ---- load x^T for the row pair: [128, (cc4, r2, q256)] ----
                xx = wp.tile([128, 2048], f16, tag="xx")
                xx4 = xx[:, :].rearrange("p (cc r q) -> p cc r q", cc=4, r=2)
                for r_ in range(2):
                    nc.sync.dma_start(
                        xx4[:, :, r_, :],
                        x_d[s0 + r_].rearrange("(cc p) q -> p cc q", p=128))
                if sp == 0:
                    # constants AFTER the first x tiles, in order of need
                    # (the SDMA rings drain FIFO; row 0's projections only
                    # need wq/wk + x)
                    nc.sync.dma_start(wq_t[:, :], wq_d[:, :])
                    nc.sync.dma_start(wk_t[:, :], wk_d[:, :])
                    nc.sync.dma_start(wv_t[:, :], wv_d[:, :])
                    nc.sync.dma_start(wg_t[:, :], wg_d[:, :])
                    nc.sync.dma_start(bgt_t[:, :], bgt_d[:, :])
                    nc.sync.dma_start(mask_t[:, :], mask_d[:, :])
                    nc.sync.dma_start(expb_t[:, :], expb_d[:, :])
                    nc.sync.dma_start(ones32_t[:, :], ones32_d[:, :])
                    nc.sync.dma_start(wo_t[:, :], wo_d[:, :])
                    nc.sync.dma_start(bo_t[:, :], bo_d[:, :])
                    nc.sync.dma_start(ones1_t[:, :], ones1_d[:, :])

                # ---- 2-row q/k projections: out [(hh,d), (tc, r, q)] ----
                qt = wp.tile([128, 1024], f16, tag="qt")  # (tc, r, q)
                kt = wp.tile([128, 1024], f16, tag="kt")
                for w_t, dst, kv, tg in ((wq_t, qt, 0, "q"), (wk_t, kt, 1, "k")):
                    for tc_ in range(2):
                        ps = pps.tile([128, 512], f32, tag="pp",
                                      name=f"{tg}{sp}_{tc_}")
                        for cc in range(2):
                            nc.tensor.matmul(
                                ps[:, :],
                                w_t[:, cc * 256 + tc_ * 128:
                                    cc * 256 + tc_ * 128 + 128],
                                xx[:, kv * 1024 + cc * 512:
                                   kv * 1024 + (cc + 1) * 512],
                                start=(cc == 0), stop=(cc == 1))
                        nc.vector.tensor_copy(
                            dst[:, tc_ * 512:(tc_ + 1) * 512], ps[:, :])

                for r in range(2):
                    s = s0 + r

                    # v natural: out[k(kc-blk), (h,d)] = xkv^T[c,k]^T @ Wv^T
                    v_ps = pps.tile([128, 512], f32, tag="pp", name=f"v{s}")
                    for kc in range(2):
                        for cc in range(2):
                            nc.tensor.matmul(
                                v_ps[:, kc * 256:(kc + 1) * 256],
                                xx[:, 1024 + cc * 512 + r * 256 + kc * 128:
                                   1024 + cc * 512 + r * 256 + kc * 128 + 128],
                                wv_t[:, cc * 256:(cc + 1) * 256],
                                start=(cc == 0), stop=(cc == 1))
                    v_sb = wp.tile([128, 512], bf16, tag="v")
                    nc.vector.tensor_copy(v_sb[:, :], v_ps[:, :])

                    # gT; sigmoid = 0.5*(tanh((g+bg)/2)+1), 0.5 in Wo
                    gt_ps = pps.tile([128, 512], f32, tag="pp", name=f"g{s}")
                    for tc_ in range(2):
                        for cc in range(2):
                            nc.tensor.matmul(
                                gt_ps[:, tc_ * 256:(tc_ + 1) * 256],
                                wg_t[:, cc * 256 + tc_ * 128:
                                     cc * 256 + tc_ * 128 + 128],
                                xx[:, cc * 512 + r * 256:
                                   cc * 512 + r * 256 + 256],
                                start=(cc == 0), stop=(cc == 1))
                    gs = wp.tile([128, 512], f32, tag="gs")
                    if bg_const is not None:
                        # bg constant -> bgt[:, 0] == 0.5*bg everywhere;
                        # one FD=512 ACTIVATE instead of two
                        nc.scalar.activation(
                            gs[:, :], gt_ps[:, :], AF.Tanh,
                            bias=bgt_t[:, 0:1], scale=0.5)
                    else:
                        for tc_ in range(2):
                            nc.scalar.activation(
                                gs[:, tc_ * 256:(tc_ + 1) * 256],
                                gt_ps[:, tc_ * 256:(tc_ + 1) * 256],
                                AF.Tanh, bias=bgt_t[:, tc_:tc_ + 1], scale=0.5)

                    # expS/A free layout: (kc, hh, tc, q); head h = 4*tc+hh
                    expS = wp.tile([128, 4096], bf16, tag="expS")
                    A = wp.tile([128, 4096], bf16, tag="A")
                    o_ps = pso.tile([128, 512], f32, tag="o", name=f"o{s}")
                    z_ps = psz.tile([128, 512], f32, tag="z", name=f"z{s}")

                    for kc in range(2):
                        # scores: 4x row-tiled over hh bands; two 2-bank
                        # tiles (hh01/hh23); 4 concurrent tiles = 4 banks
                        scA = psc.tile([128, 1024], f32, tag="sc",
                                       name=f"scA{s}_{kc}")
                        scB = psc.tile([128, 1024], f32, tag="sc",
                                       name=f"scB{s}_{kc}")
                        for tc_ in range(2):
                            for hh in range(4):
                                t, hi = (scA, hh) if hh < 2 else (scB, hh - 2)
                                nc.tensor.matmul(
                                    t[:, hi * 512 + tc_ * 256:
                                      hi * 512 + tc_ * 256 + 256],
                                    kt[hh * 32:hh * 32 + 32,
                                       tc_ * 512 + r * 256 + kc * 128:
                                       tc_ * 512 + r * 256 + kc * 128 + 128],
                                    qt[hh * 32:hh * 32 + 32,
                                       tc_ * 512 + r * 256:
                                       tc_ * 512 + r * 256 + 256],
                                    start=True, stop=True,
                                    tile_position=(hh * 32, 0))
                        # exp(s + mask_kc) per half-tile; A = expS*exp(pair)
                        for half, t in ((0, scA), (1, scB)):
                            nc.scalar.activation(
                                expS[:, kc * 2048 + half * 1024:
                                     kc * 2048 + half * 1024 + 1024],
                                t[:, :], AF.Exp,
                                bias=mask_t[:, kc * s_loc + s:
                                            kc * s_loc + s + 1])
                            nc.vector.tensor_mul(
                                A[:, kc * 2048 + half * 1024:
                                  kc * 2048 + half * 1024 + 1024],
                                expS[:, kc * 2048 + half * 1024:
                                     kc * 2048 + half * 1024 + 1024],
                                expb_t[:, kc * 2048 + half * 1024:
                                       kc * 2048 + half * 1024 + 1024])

                    # AV, 4x column-tiled over hh; out [(hh,d), (tc,q)];
                    # kc inner (one pending accumulation group per bank)
                    for tc_ in range(2):
                        for hh in range(4):
                            h = 4 * tc_ + hh
                            for kc in range(2):
                                nc.tensor.matmul(
                                    o_ps[hh * 32:hh * 32 + 32,
                                         tc_ * 256:(tc_ + 1) * 256],
                                    v_sb[:, kc * 256 + h * 32:
                                         kc * 256 + h * 32 + 32],
                                    A[:, kc * 2048 + hh * 512 + tc_ * 256:
                                       kc * 2048 + hh * 512 + tc_ * 256 + 256],
                                    start=(kc == 0), stop=(kc == 1),
                                    tile_position=(0, hh * 32))
                    # Z = sum_k A (ones lhsT), N=512 per (hh, kc)
                    for hh in range(4):
                        for kc in range(2):
                            nc.tensor.matmul(
                                z_ps[hh * 32:hh * 32 + 32, 0:512],
                                ones32_t[:, :],
                                A[:, kc * 2048 + hh * 512:
                                   kc * 2048 + hh * 512 + 512],
                                start=(kc == 0), stop=(kc == 1),
                                tile_position=(0, hh * 32))

                    # ---- normalize + gate: og = oT * (1/Z) * (gs+1) ----
                    rz = wp.tile([128, 512], f32, tag="rz")
                    nc.vector.reciprocal_approx_fast(rz[:, :], z_ps[:, :])
                    gz = wp.tile([128, 512], f32, tag="gz")
                    nc.vector.scalar_tensor_tensor(
                        gz[:, :], gs[:, :], 1.0, rz[:, :],
                        op0=ALU.add, op1=ALU.mult)
                    og = wp.tile([128, 512], f16, tag="og")
                    nc.vector.tensor_mul(og[:, :], o_ps[:, :], gz[:, :])

                    # ---- final projection y[q,(qc,c)] = og^T @ Wo^T + bo ----
                    y_ps = pps.tile([128, 512], f32, tag="pp", name=f"y{s}")
                    for qc in range(2):
                        for tc_ in range(2):
                            nc.tensor.matmul(
                                y_ps[:, qc * 256:(qc + 1) * 256],
                                og[:, tc_ * 256 + qc * 128:
                                   tc_ * 256 + qc * 128 + 128],
                                wo_t[:, tc_ * 256:(tc_ + 1) * 256],
                                start=(tc_ == 0), stop=False)
                        nc.tensor.matmul(
                            y_ps[:, qc * 256:(qc + 1) * 256],
                            ones1_t[:, :], bo_t[:, :],
                            start=False, stop=True)
                    y_sb = wp.tile([128, 512], f16, tag="y")
                    nc.scalar.copy(y_sb[:, :], y_ps[:, :])
                    nc.sync.dma_start(
                        out_d[s].rearrange("(qc p) c -> p qc c", p=128),
                        y_sb[:, :].rearrange("p (qc c) -> p qc c", qc=2))

    nc.compile()
    return nc


def get_program(s_loc=S_LOC, bg_const=None):
    key = (s_loc, bg_const, os.environ.get('KDTYPE', 'fp16'))
    if key not in _CACHE:
        _CACHE[key] = _build_program(s_loc, bg_const)
    return _CACHE[key]


def prep_inputs(q_x, kv_x, bias_mask, bias_pair, Wq, Wk, Wv, Wg, bg, Wo, bo,
                s_loc=S_LOC, n_cores=N_CORES):
    """Host-side layout prep. Returns per-core in_maps."""
    bf16 = ml_dtypes.bfloat16

    def wprep(wt):  # (C_in, T_out) -> [p, (cc, t)]
        return np.ascontiguousarray(
            wt.reshape(2, 128, 256).transpose(1, 0, 2).reshape(128, 512)
        ).astype(_mmdt())

    wq_h = wprep(np.asarray(Wq).T)     # lhsT[c, t] = Wq[t, c]
    wk_h = wprep(np.asarray(Wk).T)
    wv_h = wprep(np.asarray(Wv).T)     # rhs[c, t]
    wg_h = wprep(np.asarray(Wg).T)
    # rhs[t, c] = Wo[c, t] * 0.5 (sigmoid-tanh fold)
    wo_h = np.ascontiguousarray(
        (np.asarray(Wo).T * 0.5).reshape(2, 128, 256).transpose(1, 0, 2)
        .reshape(128, 512)).astype(_mmdt())
    # bgT[p, tc] = 0.5*bg[tc*128 + p] (ACT bias; tanh((g+bg)/2))
    bgt_h = np.ascontiguousarray(
        0.5 * np.asarray(bg, np.float32).reshape(2, 128).T)
    bo_h = np.asarray(bo, _mmdt()).reshape(1, 256)

    # expb[p, (kc, hh, tc, q)] = exp(pair[h=4*tc+hh, q, k=kc*128+p])
    eb = np.exp(np.asarray(bias_pair[0, 0], np.float64)).astype(np.float32)
    ebT = eb.transpose(0, 2, 1)  # (H, K, Q)
    expb_h = np.ascontiguousarray(
        ebT.reshape(2, 4, 2, 128, Q).transpose(3, 2, 1, 0, 4).reshape(128, 4096)
    ).astype(bf16)

    x_all = np.concatenate([
        np.asarray(q_x[0], _mmdt()).transpose(0, 2, 1),
        np.asarray(kv_x[0], _mmdt()).transpose(0, 2, 1)], axis=1)
    x_all = np.ascontiguousarray(x_all)   # (S, 2C, Q): xq | xkv
    mask_all = np.asarray(bias_mask[0, :, 0, 0, :], np.float32)  # (S, K)

    in_maps = []
    for core in range(n_cores):
        lo = core * s_loc
        m = mask_all[lo:lo + s_loc]  # (s_loc, K)
        mask_h = np.ascontiguousarray(
            m.T.reshape(2, 128, s_loc).transpose(1, 0, 2).reshape(128, 2 * s_loc))
        in_maps.append({
            "x": x_all[lo:lo + s_loc],
            "maskt": mask_h,
            "expb": expb_h,
            "wq": wq_h, "wk": wk_h, "wv": wv_h, "wg": wg_h, "wo": wo_h,
            "bgt": bgt_h, "bo": bo_h,
            "ones1": np.ones((1, 128), _mmdt()),
            "ones32": np.ones((128, 32), bf16),
        })
    return in_maps


def bg_const_of(bg):
    b = np.asarray(bg, np.float32)
    return float(b.flat[0]) if np.all(b == b.flat[0]) else None


def kernel(q_x, kv_x, bias_mask, bias_pair, Wq, Wk, Wv, Wg, bg, Wo, bo):
    from concourse import bass_utils

    nc = get_program(bg_const=bg_const_of(bg))
    in_maps = prep_inputs(q_x, kv_x, bias_mask, bias_pair,
                          Wq, Wk, Wv, Wg, bg, Wo, bo)
    res = bass_utils.run_bass_kernel_spmd(
        nc, in_maps, core_ids=list(range(N_CORES)))
    out = np.concatenate([res.results[i]["out"] for i in range(N_CORES)], axis=0)
    return out.reshape(B, S, Q, C).astype(np.float32)


# revision 17
# speedup vs baseline: 1.4649x; 1.4649x over previous
"""Trainium2 Bass kernel for the sparse_attention nn.Module problem.

Strategy: data-parallel over the MSA-row dim S (S=128 -> 16 rows per core,
8 cores). All projection weights + pair bias replicated; mask bias and
activations sharded with S. No collectives.

Design (vs the 298us v1 baseline):
  - Scores matmuls (K=DH=32) use 4x PE row-tiling (tile_position=(32*hh,0)):
    the 4 heads of a head-group run concurrently in 32-row bands of the PE
    array, consuming qT/kT in their natural [(hh,d), ...] projection layout
    (no DMA remap). The 4 concurrent tiles land in 4 distinct PSUM banks,
    split as two 2-bank tiles (hh01/hh23, pool bufs=2) so exp() releases
    banks in halves and the next group's matmuls never stall on a
    whole-tile WAR.
  - AV and softmax-denominator (Z) matmuls use 4x PE column-tiling
    (tile_position=(0,32*hh)), producing o and Z TRANSPOSED: [(hh,d),(tc,q)].
    This kills the PE transpose of the gated output: og in [t,q] layout is
    directly the lhsT of the final projection. kc is the inner loop of each
    accumulation (a PSUM bank tolerates one pending accumulation group).
  - q/k projections batch TWO rows per matmul (N=512, shared weights),
    halving their LDWEIGHTS+MATMUL count (the PE is LDW-count-bound).
  - The gate is computed transposed (gT = Wg @ x^T) so bg rides as the ACT
    per-partition bias; sigmoid(x)=0.5*(tanh(x/2)+1) with 0.5 folded into Wo.
  - exp(s+mask) as FD=1024 ACTIVATEs (mask per-partition, partitions=keys);
    exp(pair) is a host-precomputed resident tile applied with bf16 2x DVE
    multiplies; 1/Z via reciprocal_approx_fast; bo via a K=1 rank-1 matmul;
    final out evicted fp16 by ScalarE; GPSIMD off the critical path.
"""

import os
import numpy as np
import ml_dtypes

def _mmdt():
    return (ml_dtypes.bfloat16 if os.environ.get('KDTYPE', 'fp16') == 'bf16'
            else np.float16)

B, S, Q, C = 1, 128, 256, 256
H, DH = 8, 32
TOT = H * DH
N_CORES = 8
S_LOC = S // N_CORES  # 16

_CACHE = {}


def _build_program(s_loc, bg_const=None):
    import concourse.bacc as bacc
    import concourse.mybir as mybir
    from concourse import tile
    from concourse.alu_op_type import AluOpType as ALU

    assert s_loc % 2 == 0
    dt = mybir.dt
    f32, bf16 = dt.float32, dt.bfloat16
    f16 = bf16 if os.environ.get('KDTYPE', 'fp16') == 'bf16' else dt.float16
    AF = mybir.ActivationFunctionType

    nc = bacc.Bacc("TRN2", target_bir_lowering=False, debug=False,
                   num_devices=N_CORES)

    x_d = nc.dram_tensor("x", [s_loc, 2 * C, Q], f16, kind="ExternalInput").ap()
    mask_d = nc.dram_tensor("maskt", [128, 2 * s_loc], f32, kind="ExternalInput").ap()
    expb_d = nc.dram_tensor("expb", [128, 2 * H * Q], bf16, kind="ExternalInput").ap()
    wq_d = nc.dram_tensor("wq", [128, 512], f16, kind="ExternalInput").ap()
    wk_d = nc.dram_tensor("wk", [128, 512], f16, kind="ExternalInput").ap()
    wv_d = nc.dram_tensor("wv", [128, 512], f16, kind="ExternalInput").ap()
    wg_d = nc.dram_tensor("wg", [128, 512], f16, kind="ExternalInput").ap()
    wo_d = nc.dram_tensor("wo", [128, 512], f16, kind="ExternalInput").ap()
    bgt_d = nc.dram_tensor("bgt", [128, 2], f32, kind="ExternalInput").ap()
    bo_d = nc.dram_tensor("bo", [1, 256], f16, kind="ExternalInput").ap()
    ones1_d = nc.dram_tensor("ones1", [1, 128], f16, kind="ExternalInput").ap()
    ones32_d = nc.dram_tensor("ones32", [128, 32], bf16, kind="ExternalInput").ap()
    out_d = nc.dram_tensor("out", [s_loc, Q, C], f16, kind="ExternalOutput").ap()

    with tile.TileContext(nc) as tc:
        with (
            tc.tile_pool(name="const", bufs=1) as cp,
            tc.tile_pool(name="work", bufs=3) as wp,
            tc.tile_pool(name="ps_small", bufs=2, space="PSUM") as pps,
            tc.tile_pool(name="ps_sc", bufs=2, space="PSUM") as psc,
            tc.tile_pool(name="ps_o", bufs=1, space="PSUM") as pso,
            tc.tile_pool(name="ps_z", bufs=1, space="PSUM") as psz,
        ):
            # ---- resident constants ----
            wq_t = cp.tile([128, 512], f16, tag="wq")
            wk_t = cp.tile([128, 512], f16, tag="wk")
            wv_t = cp.tile([128, 512], f16, tag="wv")
            wg_t = cp.tile([128, 512], f16, tag="wg")
            wo_t = cp.tile([128, 512], f16, tag="wo")
            expb_t = cp.tile([128, 2 * H * Q], bf16, tag="expb")
            mask_t = cp.tile([128, 2 * s_loc], f32, tag="mask")
            bgt_t = cp.tile([128, 2], f32, tag="bgt")
            bo_t = cp.tile([1, 256], f16, tag="bo")
            ones1_t = cp.tile([1, 128], f16, tag="ones1")
            ones32_t = cp.tile([128, 32], bf16, tag="ones32")


            for sp in range(s_loc // 2):
                s0 = 2 * sp
                # ---- load x^T for the row pair: [128, (cc4, r2, q256)] ----
                xx = wp.tile([128, 2048], f16, tag="xx")
                xx4 = xx[:, :].rearrange("p (cc r q) -> p cc r q", cc=4, r=2)
                for r_ in range(2):
                    nc.sync.dma_start(
                        xx4[:, :, r_, :],
                        x_d[s0 + r_].rearrange("(cc p) q -> p cc q", p=128))
                if sp == 0:
                    # constants AFTER the first x tiles, in order of need
                    # (the SDMA rings drain FIFO; row 0's projections only
                    # need wq/wk + x)
                    nc.sync.dma_start(wq_t[:, :], wq_d[:, :])
                    nc.sync.dma_start(wk_t[:, :], wk_d[:, :])
                    nc.sync.dma_start(wv_t[:, :], wv_d[:, :])
                    nc.sync.dma_start(wg_t[:, :], wg_d[:, :])
                    nc.sync.dma_start(bgt_t[:, :], bgt_d[:, :])
                    nc.sync.dma_start(mask_t[:, :], mask_d[:, :])
                    nc.sync.dma_start(expb_t[:, :], expb_d[:, :])
                    nc.sync.dma_start(ones32_t[:, :], ones32_d[:, :])
                    nc.sync.dma_start(wo_t[:, :], wo_d[:, :])
                    nc.sync.dma_start(bo_t[:, :], bo_d[:, :])
                    nc.sync.dma_start(ones1_t[:, :], ones1_d[:, :])

                # ---- 2-row q/k projections: out [(hh,d), (tc, r, q)] ----
                qt = wp.tile([128, 1024], f16, tag="qt")  # (tc, r, q)
                kt = wp.tile([128, 1024], f16, tag="kt")
                for w_t, dst, kv, tg in ((wq_t, qt, 0, "q"), (wk_t, kt, 1, "k")):
                    for tc_ in range(2):
                        ps = pps.tile([128, 512], f32, tag="pp",
                                      name=f"{tg}{sp}_{tc_}")
                        for cc in range(2):
                            nc.tensor.matmul(
                                ps[:, :],
                                w_t[:, cc * 256 + tc_ * 128:
                                    cc * 256 + tc_ * 128 + 128],
                                xx[:, kv * 1024 + cc * 512:
                                   kv * 1024 + (cc + 1) * 512],
                                start=(cc == 0), stop=(cc == 1))
                        nc.vector.tensor_copy(
                            dst[:, tc_ * 512:(tc_ + 1) * 512], ps[:, :])

                for r in range(2):
                    s = s0 + r

                    # v natural: out[k(kc-blk), (h,d)] = xkv^T[c,k]^T @ Wv^T
                    v_ps = pps.tile([128, 512], f32, tag="pp", name=f"v{s}")
                    for kc in range(2):
                        for cc in range(2):
                            nc.tensor.matmul(
                                v_ps[:, kc * 256:(kc + 1) * 256],
                                xx[:, 1024 + cc * 512 + r * 256 + kc * 128:
                                   1024 + cc * 512 + r * 256 + kc * 128 + 128],
                                wv_t[:, cc * 256:(cc + 1) * 256],
                                start=(cc == 0), stop=(cc == 1))
                    v_sb = wp.tile([128, 512], bf16, tag="v")
                    nc.vector.tensor_copy(v_sb[:, :], v_ps[:, :])

                    # gT; sigmoid = 0.5*(tanh((g+bg)/2)+1), 0.5 in Wo
                    gt_ps = pps.tile([128, 512], f32, tag="pp", name=f"g{s}")
                    for tc_ in range(2):
                        for cc in range(2):
                            nc.tensor.matmul(
                                gt_ps[:, tc_ * 256:(tc_ + 1) * 256],
                                wg_t[:, cc * 256 + tc_ * 128:
                                     cc * 256 + tc_ * 128 + 128],
                                xx[:, cc * 512 + r * 256:
                                   cc * 512 + r * 256 + 256],
                                start=(cc == 0), stop=(cc == 1))
                    gs = wp.tile([128, 512], f32, tag="gs")
                    if bg_const is not None:
                        # bg constant -> bgt[:, 0] == 0.5*bg everywhere;
                        # one FD=512 ACTIVATE instead of two
                        nc.scalar.activation(
                            gs[:, :], gt_ps[:, :], AF.Tanh,
                            bias=bgt_t[:, 0:1], scale=0.5)
                    else:
                        for tc_ in range(2):
                            nc.scalar.activation(
                                gs[:, tc_ * 256:(tc_ + 1) * 256],
                                gt_ps[:, tc_ * 256:(tc_ + 1) * 256],
                                AF.Tanh, bias=bgt_t[:, tc_:tc_ + 1], scale=0.5)

                    # expS/A free layout: (kc, hh, tc, q); head h = 4*tc+hh
                    expS = wp.tile([128, 4096], bf16, tag="expS")
                    A = wp.tile([128, 4096], bf16, tag="A")
                    o_ps = pso.tile([128, 512], f32, tag="o", name=f"o{s}")
                    z_ps = psz.tile([128, 512], f32, tag="z", name=f"z{s}")

                    for kc in range(2):
                        # scores: 4x row-tiled over hh bands; two 2-bank
                        # tiles (hh01/hh23); 4 concurrent tiles = 4 banks
                        scA = psc.tile([128, 1024], f32, tag="sc",
                                       name=f"scA{s}_{kc}")
                        scB = psc.tile([128, 1024], f32, tag="sc",
                                       name=f"scB{s}_{kc}")
                        for tc_ in range(2):
                            for hh in range(4):
                                t, hi = (scA, hh) if hh < 2 else (scB, hh - 2)
                                nc.tensor.matmul(
                                    t[:, hi * 512 + tc_ * 256:
                                      hi * 512 + tc_ * 256 + 256],
                                    kt[hh * 32:hh * 32 + 32,
                                       tc_ * 512 + r * 256 + kc * 128:
                                       tc_ * 512 + r * 256 + kc * 128 + 128],
                                    qt[hh * 32:hh * 32 + 32,
                                       tc_ * 512 + r * 256:
                                       tc_ * 512 + r * 256 + 256],
                                    start=True, stop=True,
                                    tile_position=(hh * 32, 0))
                        # exp(s + mask_kc) per half-tile; A = expS*exp(pair)
                        for half, t in ((0, scA), (1, scB)):
                            nc.scalar.activation(
                                expS[:, kc * 2048 + half * 1024:
                                     kc * 2048 + half * 1024 + 1024],
                                t[:, :], AF.Exp,
                                bias=mask_t[:, kc * s_loc + s:
                                            kc * s_loc + s + 1])
                            nc.vector.tensor_mul(
                                A[:, kc * 2048 + half * 1024:
                                  kc * 2048 + half * 1024 + 1024],
                                expS[:, kc * 2048 + half * 1024:
                                     kc * 2048 + half * 1024 + 1024],
                                expb_t[:, kc * 2048 + half * 1024:
                                       kc * 2048 + half * 1024 + 1024])

                    # AV, 4x column-tiled over hh; out [(hh,d), (tc,q)];
                    # kc inner (one pending accumulation group per bank)
                    for tc_ in range(2):
                        for hh in range(4):
                            h = 4 * tc_ + hh
                            for kc in range(2):
                                nc.tensor.matmul(
                                    o_ps[hh * 32:hh * 32 + 32,
                                         tc_ * 256:(tc_ + 1) * 256],
                                    v_sb[:, kc * 256 + h * 32:
                                         kc * 256 + h * 32 + 32],
                                    A[:, kc * 2048 + hh * 512 + tc_ * 256:
                                       kc * 2048 + hh * 512 + tc_ * 256 + 256],
                                    start=(kc == 0), stop=(kc == 1),
                                    tile_position=(0, hh * 32))
                    # Z = sum_k A (ones lhsT), N=512 per (hh, kc)
                    for hh in range(4):
                        for kc in range(2):
                            nc.tensor.matmul(
                                z_ps[hh * 32:hh * 32 + 32, 0:512],
                                ones32_t[:, :],
                                A[:, kc * 2048 + hh * 512:
                                   kc * 2048 + hh * 512 + 512],
                                start=(kc == 0), stop=(kc == 1),
                                tile_position=(0, hh * 32))

                    # ---- normalize + gate: og = oT * (1/Z) * (gs+1) ----
                    rz = wp.tile([128, 512], f32, tag="rz")
                    nc.vector.reciprocal_approx_fast(rz[:, :], z_ps[:, :])
                    gz = wp.tile([128, 512], f32, tag="gz")
                    nc.vector.scalar_tensor_tensor(
                        gz[:, :], gs[:, :], 1.0, rz[:, :],
                        op0=ALU.add, op1=ALU.mult)
                    og = wp.tile([128, 512], f16, tag="og")
                    nc.vector.tensor_mul(og[:, :], o_ps[:, :], gz[:, :])

                    # ---- final projection y[q,(qc,c)] = og^T @ Wo^T + bo ----
                    y_ps = psz.tile([128, 512], f32, tag="z", name=f"y{s}")
                    for qc in range(2):
                        for tc_ in range(2):
                            nc.tensor.matmul(
                                y_ps[:, qc * 256:(qc + 1) * 256],
                                og[:, tc_ * 256 + qc * 128:
                                   tc_ * 256 + qc * 128 + 128],
                                wo_t[:, tc_ * 256:(tc_ + 1) * 256],
                                start=(tc_ == 0), stop=False)
                        nc.tensor.matmul(
                            y_ps[:, qc * 256:(qc + 1) * 256],
                            ones1_t[:, :], bo_t[:, :],
                            start=False, stop=True)
                    y_sb = wp.tile([128, 512], f16, tag="y")
                    nc.scalar.copy(y_sb[:, :], y_ps[:, :])
                    nc.sync.dma_start(
                        out_d[s].rearrange("(qc p) c -> p qc c", p=128),
                        y_sb[:, :].rearrange("p (qc c) -> p qc c", qc=2))

    nc.compile()
    return nc


def get_program(s_loc=S_LOC, bg_const=None):
    key = (s_loc, bg_const, os.environ.get('KDTYPE', 'fp16'))
    if key not in _CACHE:
        _CACHE[key] = _build_program(s_loc, bg_const)
    return _CACHE[key]


def prep_inputs(q_x, kv_x, bias_mask, bias_pair, Wq, Wk, Wv, Wg, bg, Wo, bo,
                s_loc=S_LOC, n_cores=N_CORES):
    """Host-side layout prep. Returns per-core in_maps."""
    bf16 = ml_dtypes.bfloat16

    def wprep(wt):  # (C_in, T_out) -> [p, (cc, t)]
        return np.ascontiguousarray(
            wt.reshape(2, 128, 256).transpose(1, 0, 2).reshape(128, 512)
        ).astype(_mmdt())

    wq_h = wprep(np.asarray(Wq).T)     # lhsT[c, t] = Wq[t, c]
    wk_h = wprep(np.asarray(Wk).T)
    wv_h = wprep(np.asarray(Wv).T)     # rhs[c, t]
    wg_h = wprep(np.asarray(Wg).T)
    # rhs[t, c] = Wo[c, t] * 0.5 (sigmoid-tanh fold)
    wo_h = np.ascontiguousarray(
        (np.asarray(Wo).T * 0.5).reshape(2, 128, 256).transpose(1, 0, 2)
        .reshape(128, 512)).astype(_mmdt())
    # bgT[p, tc] = 0.5*bg[tc*128 + p] (ACT bias; tanh((g+bg)/2))
    bgt_h = np.ascontiguousarray(
        0.5 * np.asarray(bg, np.float32).reshape(2, 128).T)
    bo_h = np.asarray(bo, _mmdt()).reshape(1, 256)

    # expb[p, (kc, hh, tc, q)] = exp(pair[h=4*tc+hh, q, k=kc*128+p])
    eb = np.exp(np.asarray(bias_pair[0, 0], np.float64)).astype(np.float32)
    ebT = eb.transpose(0, 2, 1)  # (H, K, Q)
    expb_h = np.ascontiguousarray(
        ebT.reshape(2, 4, 2, 128, Q).transpose(3, 2, 1, 0, 4).reshape(128, 4096)
    ).astype(bf16)

    x_all = np.concatenate([
        np.asarray(q_x[0], _mmdt()).transpose(0, 2, 1),
        np.asarray(kv_x[0], _mmdt()).transpose(0, 2, 1)], axis=1)
    x_all = np.ascontiguousarray(x_all)   # (S, 2C, Q): xq | xkv
    mask_all = np.asarray(bias_mask[0, :, 0, 0, :], np.float32)  # (S, K)

    in_maps = []
    for core in range(n_cores):
        lo = core * s_loc
        m = mask_all[lo:lo + s_loc]  # (s_loc, K)
        mask_h = np.ascontiguousarray(
            m.T.reshape(2, 128, s_loc).transpose(1, 0, 2).reshape(128, 2 * s_loc))
        in_maps.append({
            "x": x_all[lo:lo + s_loc],
            "maskt": mask_h,
            "expb": expb_h,
            "wq": wq_h, "wk": wk_h, "wv": wv_h, "wg": wg_h, "wo": wo_h,
            "bgt": bgt_h, "bo": bo_h,
            "ones1": np.ones((1, 128), _mmdt()),
            "ones32": np.ones((128, 32), bf16),
        })
    return in_maps


def bg_const_of(bg):
    b = np.asarray(bg, np.float32)
    return float(b.flat[0]) if np.all(b == b.flat[0]) else None


def kernel(q_x, kv_x, bias_mask, bias_pair, Wq, Wk, Wv, Wg, bg, Wo, bo):
    from concourse import bass_utils

    nc = get_program(bg_const=bg_const_of(bg))
    in_maps = prep_inputs(q_x, kv_x, bias_mask, bias_pair,
                          Wq, Wk, Wv, Wg, bg, Wo, bo)
    res = bass_utils.run_bass_kernel_spmd(
        nc, in_maps, core_ids=list(range(N_CORES)))
    out = np.concatenate([res.results[i]["out"] for i in range(N_CORES)], axis=0)
    return out.reshape(B, S, Q, C).astype(np.float32)
